# revision 93
# baseline (speedup 1.0000x reference)
"""Trainium2 Bass kernel for nn_DerivNet2D_v2 (quadratic-feature MLP fwd + 2
directional derivatives).

Math (per sample n, feature-major orientation):
  h1 = W4 @ [x0^2; x1^2; x0; x1] + b1          (1024, nx)
  z1 = tanh(h1);  z1sq = z1^2
  h2 = w2 @ z1sq + b2;  z2 = tanh(h2);  z2sq = z2^2
  y  = w3 @ z2sq + b3                           (1, nx)

  Derivative chain restructured so both directions share one backward matmul:
    s1 = (z2^2 - 1) * z2                       (= -z2*(1-z2^2))
    v  = wv' @ s1  with wv'[j,i] = w2[j,i] * (-4*w3[j])   (w3 scale folded
         into the host-packed v weights -> G never materialized on device)
    qt = (z1^2 - 1) * z1  (= -q);  qv = qt * v  (= q*v up to sign)
    dydx_k[n] = x_k[n] * sum_i(w1[i,k] qv[i,n]) + sum_i(w1_2[i,k] qv[i,n])
  Signs folded into the tiny constant matmuls at the end.
  Output = (y, dydx2, -dydx1).

Schedule (software pipeline, v one chunk behind mm2): per iteration ch
  f(ch-2)           - at the top, same full-128 mode as the preceding jt7
  mm1(ch) group A   - 4 concurrent row-tiled K=4 matmuls (kt 0-3)
  v(ch-1) it 0-7    - backward matmuls; its 0/1 emitted as a pair with
                      their jt7 matmuls deferred (s1_jt7 of ch-1 only
                      completes ~2.6us after the chunk boundary)
  mm1(ch) group B   - kt 4-7 (group A's PSUM banks ACT-drained by now)
  seam: y (ones-matmul, full mode, bias-add on DVE not ACT), d-span0 +
        d-span1 (col-tiled, 32-wide zero-padded weights so all sm
        partitions are written - uninit PSUM can be NaN), smS drain (DVE)
  mm2(ch) jt=0..7   - alternating kt order; s1 per jt; y-acc on DVE with
                      jt6/7's acc deferred past s1_jt7
Engine-queue discipline is the whole game at this point: the PE runs at
the 216ns/N=512 roofline with ~3us of total gaps; the ACT chain runs
~2.5us latent past each chunk boundary, so nothing that the next
iteration's PE work transitively needs may queue behind it; the DVE runs
~75% busy with z1sq/qt fused to [128,4C] ops.
PSUM: 4 banks mm1 row tiles + 3 banks big ping-pong + 1 bank d block.

Sharding: pure data-parallel over 8 cores along the batch axis; weights
replicated.
"""

import numpy as np
from contextlib import ExitStack

import concourse.bass as bass
import concourse.tile as tile
from concourse import bacc, bass_isa, mybir
from concourse.bass_utils import run_bass_kernel_spmd

F32 = mybir.dt.float32
AF = mybir.ActivationFunctionType
ALU = mybir.AluOpType

NX = 32768
N_IN = 2
H = 1024
N_CORES = 8
NXL = NX // N_CORES  # 4096 per core
JT = H // 128        # 8 feature tiles of 128

FWD_DT = mybir.dt.float16
BWD_DT = mybir.dt.float16


def build_program(nxl: int, C: int):
    """Build the per-core Bass/Tile program. Returns nc."""
    nch = nxl // C
    nc = bacc.Bacc("TRN2", target_bir_lowering=False, debug=False,
                   enable_asserts=False)

    # ---- DRAM I/O ----
    # xr layout: [wh1 (H cols) | x-features (nxl cols)] where the feature
    # rows are [x0^2; x1^2; x0; x1]. Weights-first makes [weights + chunk-0
    # data] one CONTIGUOUS 12KB range per row group: a tiny first DMA per
    # group lands ~1.3us and unblocks all of chunk 0's mm1, while the
    # 310KB remainders (first needed ~28us) follow at the throttled issue
    # cadence.
    xr = nc.dram_tensor("xr", (4, nxl + H), FWD_DT, kind="ExternalInput").ap()
    xq = nc.dram_tensor("xq", (2, nxl), BWD_DT, kind="ExternalInput").ap()
    wh2 = nc.dram_tensor("wh2", (128, JT * H), FWD_DT, kind="ExternalInput").ap()
    wv = nc.dram_tensor("wv", (128, JT * H), BWD_DT, kind="ExternalInput").ap()
    wy = nc.dram_tensor("wy", (128, JT), F32, kind="ExternalInput").ap()
    wd = nc.dram_tensor("wd", (128, 32 * JT), BWD_DT, kind="ExternalInput").ap()
    wf = nc.dram_tensor("wf", (128, 128), BWD_DT, kind="ExternalInput").ap()
    b1t = nc.dram_tensor("b1t", (128, JT), F32, kind="ExternalInput").ap()
    b2t = nc.dram_tensor("b2t", (128, JT), F32, kind="ExternalInput").ap()
    b3t = nc.dram_tensor("b3t", (1, 1), F32, kind="ExternalInput").ap()

    outy = nc.dram_tensor("outy", (1, nxl), F32, kind="ExternalOutput").ap()
    # rows: 0 = -dydx1 (outm1), 1 = dydx2 (outd2) -- one DMA per chunk
    outdd = nc.dram_tensor("outdd", (2, nxl), F32, kind="ExternalOutput").ap()

    with tile.TileContext(nc) as tc, ExitStack() as ctx:
        # ---- persistent weight tiles ----
        wpool = ctx.enter_context(tc.tile_pool(name="weights", bufs=1))
        s_wh2 = wpool.tile([128, JT * H], FWD_DT, tag="wh2")
        s_wv = wpool.tile([128, JT * H], BWD_DT, tag="wv")
        s_wy = wpool.tile([128, JT], F32, tag="wy")
        s_wd = wpool.tile([128, 32 * JT], BWD_DT, tag="wd")
        s_wf = wpool.tile([128, 128], BWD_DT, tag="wf")
        s_b1 = wpool.tile([128, JT], F32, tag="b1")
        s_b2 = wpool.tile([128, JT], F32, tag="b2")
        s_b3 = wpool.tile([1, 1], F32, tag="b3")

        # prewarm: load the ACT tanh table off the critical path
        warm = wpool.tile([128, 16], F32, tag="warm")
        nc.gpsimd.memset(warm[:], 0.0)
        nc.scalar.activation(warm[:], warm[:], AF.Tanh)
        # ones block: K=128 reduction for y as a full (128,128)-mode matmul
        s_ones = wpool.tile([128, 128], FWD_DT, tag="ones")
        nc.gpsimd.memset(s_ones[:], 1.0)
        # warmup-matmul operand: memset BEFORE the gpsimd DMA issues below so
        # the PE prewarm can start at ~0.5us
        wtile = wpool.tile([128, C], FWD_DT, tag="warmw")
        nc.gpsimd.memset(wtile[:], 0.0)

        # x+wh1 tile: [xr | wh1] replicated into the four row groups
        # (row-tiled mm1 reads both operands from SBUF partitions
        # 32g..32g+3); xx2 = [x0; x1] rows for the post-f derivative finish.
        r4_rep = wpool.tile([128, nxl + H], FWD_DT, tag="r4_rep")
        s_xx2 = wpool.tile([2, nxl], BWD_DT, tag="xx2")
        # Input DMA order: mm1's inputs (r4, wh1, b1) first, split across
        # the sync and scalar issue queues (per-queue issue cadence is the
        # early bottleneck, ~0.7-1.4us per dma_start); wh2 per-jt slices
        # behind them on sync; wv (2MB, first used ~25us) deferred to a
        # scalar-queue position after chunk-0 mm1 ACTs. DMA traffic does
        # not slow the PE; PE idle/sparse windows resetting the HAM clock
        # ramp is what hurts.
        HC = H + C
        nc.sync.dma_start(r4_rep[0:4, :HC], xr[:, :HC])
        nc.sync.dma_start(r4_rep[32:36, :HC], xr[:, :HC])
        nc.scalar.dma_start(r4_rep[64:68, :HC], xr[:, :HC])
        nc.scalar.dma_start(r4_rep[96:100, :HC], xr[:, :HC])
        nc.gpsimd.dma_start(s_b1[:], b1t[:])
        nc.gpsimd.dma_start(s_b2[:], b2t[:])
        nc.sync.dma_start(s_wh2[:, :4 * H], wh2[:, :4 * H])
        nc.scalar.dma_start(s_wh2[:, 4 * H:], wh2[:, 4 * H:])
        nc.sync.dma_start(r4_rep[0:4, HC:], xr[:, HC:])
        nc.sync.dma_start(r4_rep[32:36, HC:], xr[:, HC:])
        nc.scalar.dma_start(r4_rep[64:68, HC:], xr[:, HC:])
        nc.scalar.dma_start(r4_rep[96:100, HC:], xr[:, HC:])
        nc.gpsimd.dma_start(s_xx2[:], xq[:])
        nc.gpsimd.dma_start(s_wy[:], wy[:])
        nc.gpsimd.dma_start(s_b3[:], b3t[:])
        nc.gpsimd.dma_start(s_wd[:], wd[:])
        nc.gpsimd.dma_start(s_wf[:], wf[:])

        def issue_wv_loads():
            # scalar-queue position: after chunk-0 mm1 ACTs (~6us); first v
            # use is ~25us.
            nc.scalar.dma_start(s_wv[:, :4 * H], wv[:, :4 * H])
            nc.scalar.dma_start(s_wv[:, 4 * H:], wv[:, 4 * H:])

        # ---- pools ----
        # PSUM: 8 banks total = 4 (mm1 row tiles) + 2 (big ping-pong) + 2 (small)
        p_mm1 = ctx.enter_context(tc.tile_pool(name="mm1ps", bufs=4, space="PSUM"))
        p_big = ctx.enter_context(tc.tile_pool(name="bigps", bufs=3, space="PSUM"))
        p_sm = ctx.enter_context(tc.tile_pool(name="smps", bufs=1, space="PSUM"))
        # SBUF
        p_z1 = ctx.enter_context(tc.tile_pool(name="z1", bufs=2))
        p_z1sq = ctx.enter_context(tc.tile_pool(name="z1sq", bufs=2))
        p_qt = ctx.enter_context(tc.tile_pool(name="qt", bufs=2))
        p_s1 = ctx.enter_context(tc.tile_pool(name="s1", bufs=2))
        p_qv = ctx.enter_context(tc.tile_pool(name="qv", bufs=2))
        p_zt = ctx.enter_context(tc.tile_pool(name="zt", bufs=4))
        p_acc = ctx.enter_context(tc.tile_pool(name="acc", bufs=2))
        p_a16 = ctx.enter_context(tc.tile_pool(name="a16", bufs=2))
        p_sml = ctx.enter_context(tc.tile_pool(name="sml", bufs=3))
        p_smS = ctx.enter_context(tc.tile_pool(name="smS", bufs=2))
        p_yr = ctx.enter_context(tc.tile_pool(name="yr", bufs=2))

        # PE clock prewarm (HAM holds PE at 1.2 GHz until ~3.4us busy) +
        # filler while input DMAs land.
        def warmup(n):
            psw = p_big.tile([128, C], F32, tag="big")
            for _ in range(n):
                nc.tensor.matmul(psw[:], wtile[:, 0:128], wtile[:],
                                 start=True, stop=True)

        # ---------- per-phase emitters ----------
        def mm1_group(ch, grp, st, pool=None):
            """4 concurrent row-tiled K=4 matmuls (kt = 4*grp..4*grp+3).
            pool overrides the PSUM pool: chunk 0's group B otherwise
            trickles ~1.4us behind the serial z1-A ACT chain draining its
            banks; big-pool banks (last used by warmups, no readers) are
            free instantly."""
            cs = slice(H + ch * C, H + (ch + 1) * C)
            pl, tg = (pool, "big") if pool is not None else (p_mm1, "mm1")
            for g in range(4):
                kt = 4 * grp + g
                ps = pl.tile([128, C], F32, tag=tg)
                nc.tensor.matmul(ps[:],
                                 r4_rep[32 * g:32 * g + 4,
                                        kt * 128:(kt + 1) * 128],
                                 r4_rep[32 * g:32 * g + 4, cs],
                                 start=True, stop=True,
                                 tile_position=(32 * g, 0))
                nc.scalar.activation(st["z1b"][:, kt * C:(kt + 1) * C],
                                     ps[:], AF.Tanh,
                                     bias=s_b1[:, kt:kt + 1])

        def emit_z1sq(ch, st, half):
            # fused over 4 kt tiles: DVE per-op overhead is the scarce
            # resource (the engine runs ~90% busy unfused and a chunk late)
            ks = slice(half * 4 * C, (half + 1) * 4 * C)
            nc.vector.tensor_mul(st["z1sq"][:, ks], st["z1b"][:, ks],
                                 st["z1b"][:, ks])

        def emit_qt(st, half):
            ks = slice(half * 4 * C, (half + 1) * 4 * C)
            nc.vector.scalar_tensor_tensor(st["qt"][:, ks],
                                           st["z1sq"][:, ks], 1.0,
                                           st["z1b"][:, ks],
                                           ALU.subtract, ALU.mult)

        def v_it(bst, it):
            """one backward-matmul output tile + qv."""
            isl = slice(it * C, (it + 1) * C)
            psv = p_big.tile([128, C], F32, tag="big")
            for jt in range(JT):
                nc.tensor.matmul(
                    psv[:],
                    s_wv[:, it * H + jt * 128:it * H + (jt + 1) * 128],
                    bst["s1"][:, jt * C:(jt + 1) * C],
                    start=(jt == 0), stop=(jt == JT - 1))
            nc.vector.tensor_mul(bst["qv"][:, isl], bst["qt"][:, isl], psv[:])

        def v_pair(bst, it0, it1):
            """its it0/it1 with their jt7 matmuls deferred past both jt0-6
            spans: s1_jt7 of the previous chunk completes only ~2.6us after
            the chunk boundary (TANH7->SQ7->s1 chain), so the plain order
            stalls the PE on it."""
            psvs = {}
            for it in (it0, it1):
                psv = p_big.tile([128, C], F32, tag="big")
                psvs[it] = psv
                for jt in range(JT - 1):
                    nc.tensor.matmul(
                        psv[:],
                        s_wv[:, it * H + jt * 128:it * H + (jt + 1) * 128],
                        bst["s1"][:, jt * C:(jt + 1) * C],
                        start=(jt == 0), stop=False)
            for it in (it0, it1):
                psv = psvs[it]
                nc.tensor.matmul(
                    psv[:],
                    s_wv[:, it * H + 7 * 128:it * H + 8 * 128],
                    bst["s1"][:, 7 * C:8 * C],
                    start=False, stop=True)
                isl = slice(it * C, (it + 1) * C)
                nc.vector.tensor_mul(bst["qv"][:, isl], bst["qt"][:, isl],
                                     psv[:])

        def d_span(bst, half):
            # 4 concurrent col-tiled d matmuls: group g at partitions 32g
            # accumulates it = g (half 0) and g+4 (half 1). 32-wide weight
            # tiles (28 zero cols) so every sm partition is written --
            # uninitialized PSUM can be NaN and would poison the f-matmul
            # through its zero wf coefficients.
            for g in range(4):
                it = 4 * half + g
                nc.tensor.matmul(bst["sm"][32 * g:32 * g + 32, :],
                                 s_wd[:, 32 * it:32 * (it + 1)],
                                 bst["qv"][:, it * C:(it + 1) * C],
                                 start=(half == 0), stop=(half == 1),
                                 tile_position=(0, 32 * g))

        def y_mm(bst):
            # full-width ones-matmul: (128,128) mode; output borrows an mm1
            # bank (drained by ACT ~5us earlier -> no fresh WAR gate).
            # NOTE: gpsimd partition_all_reduce instead is a net LOSS: the
            # ~5us reduce saturates SBUF ports and slows concurrent v-phase
            # matmuls by ~6us total. The bias add runs on DVE, NOT ACT (an
            # ACT op at the seam delays every mm2 TANH/SQUARE and through
            # s1_jt7 stalls the next chunk's v matmuls).
            py = p_mm1.tile([128, C], F32, tag="mm1", name="py")
            nc.tensor.matmul(py[:], s_ones[:], bst["a16"][:],
                             start=True, stop=True)
            ys = p_sml.tile([1, C], F32, tag="ys")
            nc.vector.tensor_scalar_add(ys[:], py[0:1, :], s_b3[0:1, 0:1])
            nc.sync.dma_start(outy[0:1, bst["cs"]], ys[:])

        def sm_copy(bst):
            # drain the d PSUM bank to SBUF (fp16) for the f-matmul rhs.
            # On DVE, at the seam: the ACT queue's TANH/SQUARE chain runs
            # ~3us past each chunk boundary, so anything appended there
            # stalls the next iteration-top f; the DVE front has slack
            # (s1_jt completions are ACT-gated, not queue-gated).
            smS = p_smS.tile([128, C], BWD_DT, tag="smS", name="smS")
            nc.vector.tensor_copy(smS[:], bst["sm"][:])
            bst["smS"] = smS

        def f_mm(bst):
            # K=128 f-matmul on the drained d partials: (128,128) mode
            # (wf zero-padded); pf rows 0:2 = [A1, -A2], 32:34 = [B1, -B2].
            # Runs at the iteration top (same full mode as the preceding
            # mm2 jt7 -> no array drain); borrows a big-pool bank and is
            # drained immediately by f_fin so the bank recycles for v2.
            pf = p_big.tile([128, C], F32, tag="big", name="pf")
            nc.tensor.matmul(pf[:], s_wf[:], bst["smS"][:],
                             start=True, stop=True)
            bst["pf"] = pf

        def f_fin(bst):
            # derivative finish: dd = pf[0:2] * [x0;x1] + pf[32:34]
            ft = p_sml.tile([2, C], F32, tag="fs", name="ft")
            nc.vector.tensor_mul(ft[:], bst["pf"][0:2, :],
                                 s_xx2[:, bst["cs"]])
            dd = p_sml.tile([2, C], F32, tag="fs", name="dd")
            nc.vector.tensor_add(dd[:], ft[:], bst["pf"][32:34, :])
            nc.sync.dma_start(outdd[:, bst["cs"]], dd[:])

        def mm2_jt(ch, st, jt):
            """forward h2 tile jt -> z2, z2sq, s1, y-acc."""
            js = slice(jt * C, (jt + 1) * C)
            ps = p_big.tile([128, C], F32, tag="big")
            kts = range(JT) if jt % 2 == 0 else range(JT - 1, -1, -1)
            first = True
            for kt in kts:
                nc.tensor.matmul(
                    ps[:],
                    s_wh2[:, jt * H + kt * 128:jt * H + (kt + 1) * 128],
                    st["z1sq"][:, kt * C:(kt + 1) * C],
                    start=first, stop=(kt == (JT - 1 if jt % 2 == 0 else 0)))
                first = False
            z2 = p_zt.tile([128, C], FWD_DT, tag="zt")
            nc.scalar.activation(z2[:], ps[:], AF.Tanh,
                                 bias=s_b2[:, jt:jt + 1])
            z2sq = p_zt.tile([128, C], FWD_DT, tag="zt")
            nc.scalar.activation(z2sq[:], z2[:], AF.Square)
            # s1 = (z2sq - 1) * z2
            nc.vector.scalar_tensor_tensor(st["s1"][:, js], z2sq[:], 1.0,
                                           z2[:], ALU.subtract, ALU.mult)
            # y: k-accumulation on DVE (fp32). jt6/jt7's acc ops are
            # deferred (emitted after s1_jt7) so the next chunk's first v
            # matmuls don't wait behind them in the DVE queue; y itself is
            # not consumed until the next seam.
            if jt == 0:
                st["acc"] = p_acc.tile([128, C], F32, tag="acc", name="acc")
                nc.vector.tensor_scalar_mul(st["acc"][:], z2sq[:],
                                            s_wy[:, 0:1])
            elif jt < JT - 2:
                nc.vector.scalar_tensor_tensor(
                    st["acc"][:], z2sq[:], s_wy[:, jt:jt + 1],
                    st["acc"][:], ALU.mult, ALU.add)
            else:
                st.setdefault("accdef", []).append((jt, z2sq))
            if jt == JT - 1:
                (j6, z6), (j7, z7) = st["accdef"]
                nc.vector.scalar_tensor_tensor(
                    st["acc"][:], z6[:], s_wy[:, j6:j6 + 1],
                    st["acc"][:], ALU.mult, ALU.add)
                st["a16"] = p_a16.tile([128, C], FWD_DT, tag="a16", name="a16")
                nc.vector.scalar_tensor_tensor(
                    st["a16"][:], z7[:], s_wy[:, j7:j7 + 1],
                    st["acc"][:], ALU.mult, ALU.add)

        # ---------- main pipeline ----------
        warmup(5)
        prev = None  # back-state of chunk ch-1
        fst = None   # back-state of chunk ch-2 awaiting its f-matmul
        for ch in range(nch + 1):
            lo = ch < nch
            if fst is not None:
                # f for chunk ch-2 at the iteration top: its s1_jt7 chain
                # completed long ago, and the ~0.4us it occupies delays v0
                # past the previous chunk's s1_jt7 completion (the former
                # once-per-chunk ~0.3us PE stall).
                f_mm(fst)
                f_fin(fst)
            elif prev is not None:
                # iteration 1 has no f yet; pad so v0(ch0) doesn't outrun
                # s1(ch0)_jt7 either
                warmup(2)
            if lo:
                st = {"cs": slice(ch * C, (ch + 1) * C)}
                st["z1b"] = p_z1.tile([128, JT * C], FWD_DT, tag="z1b", name="z1b")
                st["z1sq"] = p_z1sq.tile([128, JT * C], FWD_DT, tag="z1sq", name="z1sq")
                st["qt"] = p_qt.tile([128, JT * C], BWD_DT, tag="qt", name="qt")
                st["s1"] = p_s1.tile([128, JT * C], BWD_DT, tag="s1", name="s1")
                mm1_group(ch, 0, st)
            if ch == 0:
                # group B is on the big pool (no ACT-drain gate); small pad
                # keeps the PE streaming while the z1-A ACT chain runs
                warmup(2)
            if prev is not None:
                prev["qv"] = p_qv.tile([128, JT * C], BWD_DT, tag="qv", name="qv")
                prev["sm"] = p_sm.tile([128, C], F32, tag="sm", name="sm")
                v_pair(prev, 0, 1)
                if lo:
                    emit_z1sq(ch, st, 0)        # kt 0..3
                v_it(prev, 2)
                if lo:
                    emit_qt(st, 0)
                v_it(prev, 3)
            if lo:
                mm1_group(ch, 1, st, pool=p_big if ch == 0 else None)
            if ch == 0:
                # bridge until chunk-0's z1sq chain (ACT+DVE serial) catches
                # up, so mm2 streams without HAM-resetting stalls
                warmup(6)
            if prev is not None:
                # its 6,7 stay clean so qv6/qv7 retire with no DVE-queue
                # delay (they gate the seam's PSUM bank reuse and d-span1)
                v_it(prev, 4)
                v_it(prev, 5)
                if lo:
                    emit_z1sq(ch, st, 1)        # kt 4..7
                v_it(prev, 6)
                if lo:
                    emit_qt(st, 1)
                v_it(prev, 7)
                # seam, ordered to minimize PE mode switches (each one
                # drains the array): y continues v's full-128 mode
                # (weights can prefetch during v7), then both col-tiled
                # d spans back-to-back; qv7 lands during y+d0.
                if lo:
                    y_mm(prev)
                    d_span(prev, 0)
                    d_span(prev, 1)
                    sm_copy(prev)
                else:
                    # final drain: d + sm_copy first so the last f isn't
                    # serialized behind y; ACT is idle here and beats the
                    # DVE queue (still draining qv6/qv7)
                    d_span(prev, 0)
                    d_span(prev, 1)
                    smS = p_smS.tile([128, C], BWD_DT, tag="smS",
                                     name="smSd")
                    nc.scalar.activation(smS[:], prev["sm"][:], AF.Identity)
                    prev["smS"] = smS
                    y_mm(prev)
            if lo:
                if prev is None:
                    emit_z1sq(ch, st, 0)
                    emit_z1sq(ch, st, 1)
                    emit_qt(st, 0)
                    emit_qt(st, 1)
                mm2_jt(ch, st, 0)
                if ch == 0:
                    # scalar-queue position after all mm1 ACTs: the 2MB wv
                    # stream starts ~10us; first v use is ~35us.
                    issue_wv_loads()
                    warmup(2)
            if lo:
                for jt in range(1, JT):
                    mm2_jt(ch, st, jt)
            fst = prev
            prev = st if lo else None
        # drain the last pending f (chunk nch-1)
        if fst is not None:
            f_mm(fst)
            f_fin(fst)

    nc.compile()
    return nc


def _pack_k(m: np.ndarray) -> np.ndarray:
    """(1024, F) contraction-major -> (128, 8*F); tile kt at [:, kt*F:(kt+1)*F]."""
    kdim, f = m.shape
    assert kdim == H
    return np.ascontiguousarray(
        m.reshape(JT, 128, f).transpose(1, 0, 2).reshape(128, JT * f))


def _pack_k_outer(m: np.ndarray) -> np.ndarray:
    """(1024, 1024) contraction-major -> (128, 8*1024) with the OUTPUT tile
    index outer: tile (kt, jt) at [:, jt*1024 + kt*128]."""
    t = m.reshape(JT, 128, JT, 128).transpose(1, 2, 0, 3)  # (kp, jt, kt, jc)
    return np.ascontiguousarray(t.reshape(128, JT * H))


def _fwdcast(a: np.ndarray) -> np.ndarray:
    return a.astype(mybir.dt.np(FWD_DT))


def _bwdcast(a: np.ndarray) -> np.ndarray:
    return a.astype(mybir.dt.np(BWD_DT))


def prep_weights(w1, w1_2, b1, w2, b2, w3, b3):
    f = np.float32
    wh1 = np.ascontiguousarray(
        np.stack([w1[:, 0], w1[:, 1], w1_2[:, 0], w1_2[:, 1]]).astype(f))
    wh2 = _pack_k_outer(np.ascontiguousarray(w2.T).astype(f))  # lhsT[k,j]=w2[j,k]
    # v weights with the -4*w3[j] G-scale folded in: lhsT[j,i]=w2[j,i]*(-4 w3[j])
    wvs = (w2.astype(f) * (-4.0 * w3.reshape(-1, 1).astype(f)))
    wv = _pack_k_outer(wvs)
    wy = np.ascontiguousarray(w3.reshape(H).reshape(JT, 128).T.astype(f))
    wd4 = _pack_k(np.ascontiguousarray(
        np.stack([w1[:, 0], w1_2[:, 0], w1[:, 1], w1_2[:, 1]], axis=1)).astype(f))
    # widen each it-tile from 4 to 32 weight columns (28 zeros) so the d
    # matmuls write all 32 rows of each sm col group
    wd = np.zeros((128, 32 * JT), dtype=f)
    for it in range(JT):
        wd[:, 32 * it:32 * it + 4] = wd4[:, 4 * it:4 * (it + 1)]
    # f-matmul now consumes the raw d partials (sm): rows of sm are
    # [A1g, B1g, A2g, B2g] (A=w1-weighted, B=w1_2-weighted, per col group g).
    # pf rows 0:2 = [A1, -A2], rows 32:34 = [B1, -B2] (32-aligned partition
    # bases for the DVE finish: dd = pf[0:2]*[x0;x1] + pf[32:34]).
    wf = np.zeros((128, 128), dtype=f)
    for g in range(4):
        wf[32 * g + 0, 0] = 1.0    # A1
        wf[32 * g + 2, 1] = -1.0   # -A2
        wf[32 * g + 1, 32] = 1.0   # B1
        wf[32 * g + 3, 33] = -1.0  # -B2
    b1t = np.ascontiguousarray(b1.reshape(JT, 128).T.astype(f))
    b2t = np.ascontiguousarray(b2.reshape(JT, 128).T.astype(f))
    b3t = np.asarray(b3, dtype=f).reshape(1, 1)
    return _fwdcast(wh1), dict(wh2=_fwdcast(wh2), wv=_bwdcast(wv),
                               wy=wy, wd=_bwdcast(wd), wf=_bwdcast(wf),
                               b1t=b1t, b2t=b2t, b3t=b3t)


_PROG_CACHE: dict = {}


def _install_trace_support():
    """The agent image lacks the ``antenv.axon_hooks`` shim that the axon
    NTFF-profiling path imports; recreate it and register the ctypes hook.
    Also neuter ``upload_artifacts`` (zero-egress container)."""
    import sys
    import types
    try:
        import antenv.axon_hooks  # noqa: F401
    except ImportError:
        import antenv
        mod = types.ModuleType("antenv.axon_hooks")
        holder = {}
        mod.set_axon_ntff_profile_hook = lambda h: holder.__setitem__("h", h)
        mod.get_axon_ntff_profile_hook = lambda: holder.get("h")
        sys.modules["antenv.axon_hooks"] = mod
        antenv.axon_hooks = mod
        from trn_agent_boot.trn_boot import _ntff_profile_via_ctypes
        hook = _ntff_profile_via_ctypes("/opt/axon/libaxon_pjrt.so")
        if hook is not None:
            mod.set_axon_ntff_profile_hook(hook)
    import concourse.bass_utils as bu
    bu.upload_artifacts = lambda tmpdir: tmpdir


def kernel(x, w1, w1_2, b1, w2, b2, w3, b3, trace=False, _chunk=512):
    x = np.asarray(x, dtype=np.float32)
    wh1p, wdict = prep_weights(np.asarray(w1), np.asarray(w1_2),
                               np.asarray(b1), np.asarray(w2),
                               np.asarray(b2), np.asarray(w3),
                               np.asarray(b3))

    key = (NXL, _chunk)
    if key not in _PROG_CACHE:
        _PROG_CACHE[key] = build_program(NXL, _chunk)
    nc = _PROG_CACHE[key]

    in_maps = []
    for c in range(N_CORES):
        xs = x[c * NXL:(c + 1) * NXL]                 # (NXL, 2)
        x0, x1 = xs[:, 0].copy(), xs[:, 1].copy()
        xrs = _fwdcast(np.ascontiguousarray(np.concatenate([
            wh1p, np.stack([x0 * x0, x1 * x1, x0, x1])],
            axis=1)))                                 # (4, H + NXL)
        xqs = _bwdcast(np.ascontiguousarray(
            np.stack([x0, x1])))                      # (2, NXL)
        in_maps.append({"xr": xrs, "xq": xqs, **wdict})

    if trace:
        _install_trace_support()
    res = run_bass_kernel_spmd(nc, in_maps, core_ids=list(range(N_CORES)),
                               trace=trace)

    y = np.concatenate([res.results[c]["outy"].reshape(NXL)
                        for c in range(N_CORES)]).reshape(NX, 1)
    d2 = np.concatenate([res.results[c]["outdd"][1].reshape(NXL)
                         for c in range(N_CORES)]).reshape(NX, 1)
    m1 = np.concatenate([res.results[c]["outdd"][0].reshape(NXL)
                         for c in range(N_CORES)]).reshape(NX, 1)
    out = (y.astype(np.float32), d2.astype(np.float32), m1.astype(np.float32))
    if trace:
        return out, res
    return out



# revision 94
# speedup vs baseline: 1.0217x; 1.0217x over previous
"""Trainium2 Bass kernel for nn_DerivNet2D_v2 (quadratic-feature MLP fwd + 2
directional derivatives).

Math (per sample n, feature-major orientation):
  h1 = W4 @ [x0^2; x1^2; x0; x1] + b1          (1024, nx)
  z1 = tanh(h1);  z1sq = z1^2
  h2 = w2 @ z1sq + b2;  z2 = tanh(h2);  z2sq = z2^2
  y  = w3 @ z2sq + b3                           (1, nx)

  Derivative chain restructured so both directions share one backward matmul:
    s1 = (z2^2 - 1) * z2                       (= -z2*(1-z2^2))
    v  = wv' @ s1  with wv'[j,i] = w2[j,i] * (-4*w3[j])   (w3 scale folded
         into the host-packed v weights -> G never materialized on device)
    qt = (z1^2 - 1) * z1  (= -q);  qv = qt * v  (= q*v up to sign)
    dydx_k[n] = x_k[n] * sum_i(w1[i,k] qv[i,n]) + sum_i(w1_2[i,k] qv[i,n])
  Signs folded into the tiny constant matmuls at the end.
  Output = (y, dydx2, -dydx1).

Schedule (software pipeline, v one chunk behind mm2): per iteration ch
  f(ch-2)           - at the top, same full-128 mode as the preceding jt7
  mm1(ch) group A   - 4 concurrent row-tiled K=4 matmuls (kt 0-3)
  v(ch-1) it 0-7    - backward matmuls; its 0/1 emitted as a pair with
                      their jt7 matmuls deferred (s1_jt7 of ch-1 only
                      completes ~2.6us after the chunk boundary)
  mm1(ch) group B   - kt 4-7 (group A's PSUM banks ACT-drained by now)
  seam: y (ones-matmul, full mode, bias-add on DVE not ACT), d-span0 +
        d-span1 (col-tiled, 32-wide zero-padded weights so all sm
        partitions are written - uninit PSUM can be NaN), smS drain (DVE)
  mm2(ch) jt=0..7   - alternating kt order; s1 per jt; y-acc on DVE with
                      jt6/7's acc deferred past s1_jt7
Engine-queue discipline is the whole game at this point: the PE runs at
the 216ns/N=512 roofline with ~3us of total gaps; the ACT chain runs
~2.5us latent past each chunk boundary, so nothing that the next
iteration's PE work transitively needs may queue behind it; the DVE runs
~75% busy with z1sq/qt fused to [128,4C] ops.
PSUM: 4 banks mm1 row tiles + 3 banks big ping-pong + 1 bank d block.

Sharding: pure data-parallel over 8 cores along the batch axis; weights
replicated.
"""

import numpy as np
from contextlib import ExitStack

import concourse.bass as bass
import concourse.tile as tile
from concourse import bacc, bass_isa, mybir
from concourse.bass_utils import run_bass_kernel_spmd

F32 = mybir.dt.float32
AF = mybir.ActivationFunctionType
ALU = mybir.AluOpType

NX = 32768
N_IN = 2
H = 1024
N_CORES = 8
NXL = NX // N_CORES  # 4096 per core
JT = H // 128        # 8 feature tiles of 128

FWD_DT = mybir.dt.float16
BWD_DT = mybir.dt.float16


def build_program(nxl: int, C: int):
    """Build the per-core Bass/Tile program. Returns nc."""
    nch = nxl // C
    nc = bacc.Bacc("TRN2", target_bir_lowering=False, debug=False,
                   enable_asserts=False)

    # ---- DRAM I/O ----
    # xr carries [x0^2; x1^2; x0; x1] with the mm1 weights wh1 appended as
    # columns nxl:nxl+H, so each 32-row-group replica lands in ONE DMA
    # (early DMA issue cadence is ~1.4us per dma_start)
    xr = nc.dram_tensor("xr", (4, nxl + H), FWD_DT, kind="ExternalInput").ap()
    xq = nc.dram_tensor("xq", (2, nxl), BWD_DT, kind="ExternalInput").ap()
    wh2 = nc.dram_tensor("wh2", (128, JT * H), FWD_DT, kind="ExternalInput").ap()
    wv = nc.dram_tensor("wv", (128, JT * H), BWD_DT, kind="ExternalInput").ap()
    wy = nc.dram_tensor("wy", (128, JT), F32, kind="ExternalInput").ap()
    wd = nc.dram_tensor("wd", (128, 32 * JT), BWD_DT, kind="ExternalInput").ap()
    wf = nc.dram_tensor("wf", (128, 128), BWD_DT, kind="ExternalInput").ap()
    b1t = nc.dram_tensor("b1t", (128, JT), F32, kind="ExternalInput").ap()
    b2t = nc.dram_tensor("b2t", (128, JT), F32, kind="ExternalInput").ap()
    b3t = nc.dram_tensor("b3t", (1, 1), F32, kind="ExternalInput").ap()

    outy = nc.dram_tensor("outy", (1, nxl), F32, kind="ExternalOutput").ap()
    # rows: 0 = -dydx1 (outm1), 1 = dydx2 (outd2) -- one DMA per chunk
    outdd = nc.dram_tensor("outdd", (2, nxl), F32, kind="ExternalOutput").ap()

    with tile.TileContext(nc) as tc, ExitStack() as ctx:
        # ---- persistent weight tiles ----
        wpool = ctx.enter_context(tc.tile_pool(name="weights", bufs=1))
        s_wh2 = wpool.tile([128, JT * H], FWD_DT, tag="wh2")
        s_wv = wpool.tile([128, JT * H], BWD_DT, tag="wv")
        s_wy = wpool.tile([128, JT], F32, tag="wy")
        s_wd = wpool.tile([128, 32 * JT], BWD_DT, tag="wd")
        s_wf = wpool.tile([128, 128], BWD_DT, tag="wf")
        s_b1 = wpool.tile([128, JT], F32, tag="b1")
        s_b2 = wpool.tile([128, JT], F32, tag="b2")
        s_b3 = wpool.tile([1, 1], F32, tag="b3")

        # prewarm: load the ACT tanh table off the critical path
        warm = wpool.tile([128, 16], F32, tag="warm")
        nc.gpsimd.memset(warm[:], 0.0)
        nc.scalar.activation(warm[:], warm[:], AF.Tanh)
        # ones block: K=128 reduction for y as a full (128,128)-mode matmul
        s_ones = wpool.tile([128, 128], FWD_DT, tag="ones")
        nc.gpsimd.memset(s_ones[:], 1.0)
        # warmup-matmul operand: memset BEFORE the gpsimd DMA issues below so
        # the PE prewarm can start at ~0.5us
        wtile = wpool.tile([128, C], FWD_DT, tag="warmw")
        nc.gpsimd.memset(wtile[:], 0.0)

        # x+wh1 tile: [xr | wh1] replicated into the four row groups
        # (row-tiled mm1 reads both operands from SBUF partitions
        # 32g..32g+3); xx2 = [x0; x1] rows for the post-f derivative finish.
        r4_rep = wpool.tile([128, nxl + H], FWD_DT, tag="r4_rep")
        s_xx2 = wpool.tile([2, nxl], BWD_DT, tag="xx2")
        # Input DMA order: mm1's inputs (r4, wh1, b1) first, split across
        # the sync and scalar issue queues (per-queue issue cadence is the
        # early bottleneck, ~0.7-1.4us per dma_start); wh2 per-jt slices
        # behind them on sync; wv (2MB, first used ~25us) deferred to a
        # scalar-queue position after chunk-0 mm1 ACTs. DMA traffic does
        # not slow the PE; PE idle/sparse windows resetting the HAM clock
        # ramp is what hurts.
        nc.sync.dma_start(r4_rep[0:4, :], xr[:])
        nc.sync.dma_start(r4_rep[32:36, :], xr[:])
        nc.scalar.dma_start(r4_rep[64:68, :], xr[:])
        nc.scalar.dma_start(r4_rep[96:100, :], xr[:])
        nc.gpsimd.dma_start(s_b1[:], b1t[:])
        nc.gpsimd.dma_start(s_b2[:], b2t[:])
        nc.sync.dma_start(s_wh2[:, :4 * H], wh2[:, :4 * H])
        nc.scalar.dma_start(s_wh2[:, 4 * H:], wh2[:, 4 * H:])
        nc.gpsimd.dma_start(s_xx2[:], xq[:])
        nc.gpsimd.dma_start(s_wy[:], wy[:])
        nc.gpsimd.dma_start(s_b3[:], b3t[:])
        nc.gpsimd.dma_start(s_wd[:], wd[:])
        nc.gpsimd.dma_start(s_wf[:], wf[:])

        def issue_wv_loads():
            # scalar-queue position: after chunk-0 mm1 ACTs (~6us); first v
            # use is ~25us.
            nc.scalar.dma_start(s_wv[:, :4 * H], wv[:, :4 * H])
            nc.scalar.dma_start(s_wv[:, 4 * H:], wv[:, 4 * H:])

        # ---- pools ----
        # PSUM: 8 banks total = 4 (mm1 row tiles) + 2 (big ping-pong) + 2 (small)
        p_mm1 = ctx.enter_context(tc.tile_pool(name="mm1ps", bufs=4, space="PSUM"))
        p_big = ctx.enter_context(tc.tile_pool(name="bigps", bufs=3, space="PSUM"))
        p_sm = ctx.enter_context(tc.tile_pool(name="smps", bufs=1, space="PSUM"))
        # SBUF
        p_z1 = ctx.enter_context(tc.tile_pool(name="z1", bufs=2))
        p_z1sq = ctx.enter_context(tc.tile_pool(name="z1sq", bufs=2))
        p_qt = ctx.enter_context(tc.tile_pool(name="qt", bufs=2))
        p_s1 = ctx.enter_context(tc.tile_pool(name="s1", bufs=2))
        p_qv = ctx.enter_context(tc.tile_pool(name="qv", bufs=2))
        p_zt = ctx.enter_context(tc.tile_pool(name="zt", bufs=4))
        p_acc = ctx.enter_context(tc.tile_pool(name="acc", bufs=2))
        p_a16 = ctx.enter_context(tc.tile_pool(name="a16", bufs=2))
        p_sml = ctx.enter_context(tc.tile_pool(name="sml", bufs=3))
        p_smS = ctx.enter_context(tc.tile_pool(name="smS", bufs=2))
        p_yr = ctx.enter_context(tc.tile_pool(name="yr", bufs=2))

        # PE clock prewarm (HAM holds PE at 1.2 GHz until ~3.4us busy) +
        # filler while input DMAs land.
        def warmup(n):
            psw = p_big.tile([128, C], F32, tag="big")
            for _ in range(n):
                nc.tensor.matmul(psw[:], wtile[:, 0:128], wtile[:],
                                 start=True, stop=True)

        # ---------- per-phase emitters ----------
        def mm1_group(ch, grp, st, pool=None):
            """4 concurrent row-tiled K=4 matmuls (kt = 4*grp..4*grp+3).
            pool overrides the PSUM pool: chunk 0's group B otherwise
            trickles ~1.4us behind the serial z1-A ACT chain draining its
            banks; big-pool banks (last used by warmups, no readers) are
            free instantly."""
            cs = slice(ch * C, (ch + 1) * C)
            pl, tg = (pool, "big") if pool is not None else (p_mm1, "mm1")
            for g in range(4):
                kt = 4 * grp + g
                ps = pl.tile([128, C], F32, tag=tg)
                nc.tensor.matmul(ps[:],
                                 r4_rep[32 * g:32 * g + 4,
                                        nxl + kt * 128:nxl + (kt + 1) * 128],
                                 r4_rep[32 * g:32 * g + 4, cs],
                                 start=True, stop=True,
                                 tile_position=(32 * g, 0))
                nc.scalar.activation(st["z1b"][:, kt * C:(kt + 1) * C],
                                     ps[:], AF.Tanh,
                                     bias=s_b1[:, kt:kt + 1])

        def emit_z1sq(ch, st, half):
            # fused over 4 kt tiles: DVE per-op overhead is the scarce
            # resource (the engine runs ~90% busy unfused and a chunk late)
            ks = slice(half * 4 * C, (half + 1) * 4 * C)
            nc.vector.tensor_mul(st["z1sq"][:, ks], st["z1b"][:, ks],
                                 st["z1b"][:, ks])

        def emit_qt(st, half):
            ks = slice(half * 4 * C, (half + 1) * 4 * C)
            nc.vector.scalar_tensor_tensor(st["qt"][:, ks],
                                           st["z1sq"][:, ks], 1.0,
                                           st["z1b"][:, ks],
                                           ALU.subtract, ALU.mult)

        def v_it(bst, it):
            """one backward-matmul output tile + qv."""
            isl = slice(it * C, (it + 1) * C)
            psv = p_big.tile([128, C], F32, tag="big")
            for jt in range(JT):
                nc.tensor.matmul(
                    psv[:],
                    s_wv[:, it * H + jt * 128:it * H + (jt + 1) * 128],
                    bst["s1"][:, jt * C:(jt + 1) * C],
                    start=(jt == 0), stop=(jt == JT - 1))
            nc.vector.tensor_mul(bst["qv"][:, isl], bst["qt"][:, isl], psv[:])

        def v_pair(bst, it0, it1):
            """its it0/it1 with their jt7 matmuls deferred past both jt0-6
            spans: s1_jt7 of the previous chunk completes only ~2.6us after
            the chunk boundary (TANH7->SQ7->s1 chain), so the plain order
            stalls the PE on it."""
            psvs = {}
            for it in (it0, it1):
                psv = p_big.tile([128, C], F32, tag="big")
                psvs[it] = psv
                for jt in range(JT - 1):
                    nc.tensor.matmul(
                        psv[:],
                        s_wv[:, it * H + jt * 128:it * H + (jt + 1) * 128],
                        bst["s1"][:, jt * C:(jt + 1) * C],
                        start=(jt == 0), stop=False)
            for it in (it0, it1):
                psv = psvs[it]
                nc.tensor.matmul(
                    psv[:],
                    s_wv[:, it * H + 7 * 128:it * H + 8 * 128],
                    bst["s1"][:, 7 * C:8 * C],
                    start=False, stop=True)
                isl = slice(it * C, (it + 1) * C)
                nc.vector.tensor_mul(bst["qv"][:, isl], bst["qt"][:, isl],
                                     psv[:])

        def d_span(bst, half):
            # 4 concurrent col-tiled d matmuls: group g at partitions 32g
            # accumulates it = g (half 0) and g+4 (half 1). 32-wide weight
            # tiles (28 zero cols) so every sm partition is written --
            # uninitialized PSUM can be NaN and would poison the f-matmul
            # through its zero wf coefficients.
            for g in range(4):
                it = 4 * half + g
                nc.tensor.matmul(bst["sm"][32 * g:32 * g + 32, :],
                                 s_wd[:, 32 * it:32 * (it + 1)],
                                 bst["qv"][:, it * C:(it + 1) * C],
                                 start=(half == 0), stop=(half == 1),
                                 tile_position=(0, 32 * g))

        def y_mm(bst):
            # full-width ones-matmul: (128,128) mode; output borrows an mm1
            # bank (drained by ACT ~5us earlier -> no fresh WAR gate).
            # NOTE: gpsimd partition_all_reduce instead is a net LOSS: the
            # ~5us reduce saturates SBUF ports and slows concurrent v-phase
            # matmuls by ~6us total. The bias add runs on DVE, NOT ACT (an
            # ACT op at the seam delays every mm2 TANH/SQUARE and through
            # s1_jt7 stalls the next chunk's v matmuls).
            py = p_mm1.tile([128, C], F32, tag="mm1", name="py")
            nc.tensor.matmul(py[:], s_ones[:], bst["a16"][:],
                             start=True, stop=True)
            ys = p_sml.tile([1, C], F32, tag="ys")
            nc.vector.tensor_scalar_add(ys[:], py[0:1, :], s_b3[0:1, 0:1])
            nc.sync.dma_start(outy[0:1, bst["cs"]], ys[:])

        def sm_copy(bst):
            # drain the d PSUM bank to SBUF (fp16) for the f-matmul rhs.
            # On DVE, at the seam: the ACT queue's TANH/SQUARE chain runs
            # ~3us past each chunk boundary, so anything appended there
            # stalls the next iteration-top f; the DVE front has slack
            # (s1_jt completions are ACT-gated, not queue-gated).
            smS = p_smS.tile([128, C], BWD_DT, tag="smS", name="smS")
            nc.vector.tensor_copy(smS[:], bst["sm"][:])
            bst["smS"] = smS

        def f_mm(bst):
            # K=128 f-matmul on the drained d partials: (128,128) mode
            # (wf zero-padded); pf rows 0:2 = [A1, -A2], 32:34 = [B1, -B2].
            # Runs at the iteration top (same full mode as the preceding
            # mm2 jt7 -> no array drain); borrows a big-pool bank and is
            # drained immediately by f_fin so the bank recycles for v2.
            pf = p_big.tile([128, C], F32, tag="big", name="pf")
            nc.tensor.matmul(pf[:], s_wf[:], bst["smS"][:],
                             start=True, stop=True)
            bst["pf"] = pf

        def f_fin(bst):
            # derivative finish: dd = pf[0:2] * [x0;x1] + pf[32:34]
            ft = p_sml.tile([2, C], F32, tag="fs", name="ft")
            nc.vector.tensor_mul(ft[:], bst["pf"][0:2, :],
                                 s_xx2[:, bst["cs"]])
            dd = p_sml.tile([2, C], F32, tag="fs", name="dd")
            nc.vector.tensor_add(dd[:], ft[:], bst["pf"][32:34, :])
            nc.sync.dma_start(outdd[:, bst["cs"]], dd[:])

        def mm2_jt(ch, st, jt):
            """forward h2 tile jt -> z2, z2sq, s1, y-acc."""
            js = slice(jt * C, (jt + 1) * C)
            ps = p_big.tile([128, C], F32, tag="big")
            kts = range(JT) if jt % 2 == 0 else range(JT - 1, -1, -1)
            first = True
            for kt in kts:
                nc.tensor.matmul(
                    ps[:],
                    s_wh2[:, jt * H + kt * 128:jt * H + (kt + 1) * 128],
                    st["z1sq"][:, kt * C:(kt + 1) * C],
                    start=first, stop=(kt == (JT - 1 if jt % 2 == 0 else 0)))
                first = False
            z2 = p_zt.tile([128, C], FWD_DT, tag="zt")
            nc.scalar.activation(z2[:], ps[:], AF.Tanh,
                                 bias=s_b2[:, jt:jt + 1])
            z2sq = p_zt.tile([128, C], FWD_DT, tag="zt")
            nc.scalar.activation(z2sq[:], z2[:], AF.Square)
            # s1 = (z2sq - 1) * z2
            nc.vector.scalar_tensor_tensor(st["s1"][:, js], z2sq[:], 1.0,
                                           z2[:], ALU.subtract, ALU.mult)
            # y: k-accumulation on DVE (fp32). jt6/jt7's acc ops are
            # deferred (emitted after s1_jt7) so the next chunk's first v
            # matmuls don't wait behind them in the DVE queue; y itself is
            # not consumed until the next seam.
            if jt == 0:
                st["acc"] = p_acc.tile([128, C], F32, tag="acc", name="acc")
                nc.vector.tensor_scalar_mul(st["acc"][:], z2sq[:],
                                            s_wy[:, 0:1])
            elif jt < JT - 2:
                nc.vector.scalar_tensor_tensor(
                    st["acc"][:], z2sq[:], s_wy[:, jt:jt + 1],
                    st["acc"][:], ALU.mult, ALU.add)
            else:
                st.setdefault("accdef", []).append((jt, z2sq))
            if jt == JT - 1:
                (j6, z6), (j7, z7) = st["accdef"]
                nc.vector.scalar_tensor_tensor(
                    st["acc"][:], z6[:], s_wy[:, j6:j6 + 1],
                    st["acc"][:], ALU.mult, ALU.add)
                st["a16"] = p_a16.tile([128, C], FWD_DT, tag="a16", name="a16")
                nc.vector.scalar_tensor_tensor(
                    st["a16"][:], z7[:], s_wy[:, j7:j7 + 1],
                    st["acc"][:], ALU.mult, ALU.add)

        # ---------- main pipeline ----------
        warmup(13)
        prev = None  # back-state of chunk ch-1
        fst = None   # back-state of chunk ch-2 awaiting its f-matmul
        for ch in range(nch + 1):
            lo = ch < nch
            if fst is not None:
                # f for chunk ch-2 at the iteration top: its s1_jt7 chain
                # completed long ago, and the ~0.4us it occupies delays v0
                # past the previous chunk's s1_jt7 completion (the former
                # once-per-chunk ~0.3us PE stall).
                f_mm(fst)
                f_fin(fst)
            elif prev is not None:
                # iteration 1 has no f yet; pad so v0(ch0) doesn't outrun
                # s1(ch0)_jt7 either
                warmup(2)
            if lo:
                st = {"cs": slice(ch * C, (ch + 1) * C)}
                st["z1b"] = p_z1.tile([128, JT * C], FWD_DT, tag="z1b", name="z1b")
                st["z1sq"] = p_z1sq.tile([128, JT * C], FWD_DT, tag="z1sq", name="z1sq")
                st["qt"] = p_qt.tile([128, JT * C], BWD_DT, tag="qt", name="qt")
                st["s1"] = p_s1.tile([128, JT * C], BWD_DT, tag="s1", name="s1")
                mm1_group(ch, 0, st)
            if ch == 0:
                # cover group A's ACT drain (mm1 B reuses its PSUM banks)
                warmup(6)
            if prev is not None:
                prev["qv"] = p_qv.tile([128, JT * C], BWD_DT, tag="qv", name="qv")
                prev["sm"] = p_sm.tile([128, C], F32, tag="sm", name="sm")
                v_pair(prev, 0, 1)
                if lo:
                    emit_z1sq(ch, st, 0)        # kt 0..3
                v_it(prev, 2)
                if lo:
                    emit_qt(st, 0)
                v_it(prev, 3)
            if lo:
                mm1_group(ch, 1, st, pool=p_big if ch == 0 else None)
            if ch == 0:
                # bridge until chunk-0's z1sq chain (ACT+DVE serial) catches
                # up, so mm2 streams without HAM-resetting stalls
                warmup(4)
            if prev is not None:
                # its 6,7 stay clean so qv6/qv7 retire with no DVE-queue
                # delay (they gate the seam's PSUM bank reuse and d-span1)
                v_it(prev, 4)
                v_it(prev, 5)
                if lo:
                    emit_z1sq(ch, st, 1)        # kt 4..7
                v_it(prev, 6)
                if lo:
                    emit_qt(st, 1)
                v_it(prev, 7)
                # seam, ordered to minimize PE mode switches (each one
                # drains the array): y continues v's full-128 mode
                # (weights can prefetch during v7), then both col-tiled
                # d spans back-to-back; qv7 lands during y+d0.
                if lo:
                    y_mm(prev)
                    d_span(prev, 0)
                    d_span(prev, 1)
                    sm_copy(prev)
                else:
                    # final drain: d + sm_copy first so the last f isn't
                    # serialized behind y; ACT is idle here and beats the
                    # DVE queue (still draining qv6/qv7)
                    d_span(prev, 0)
                    d_span(prev, 1)
                    smS = p_smS.tile([128, C], BWD_DT, tag="smS",
                                     name="smSd")
                    nc.scalar.activation(smS[:], prev["sm"][:], AF.Identity)
                    prev["smS"] = smS
                    y_mm(prev)
            if lo:
                if prev is None:
                    emit_z1sq(ch, st, 0)
                    emit_z1sq(ch, st, 1)
                    emit_qt(st, 0)
                    emit_qt(st, 1)
                mm2_jt(ch, st, 0)
                if ch == 0:
                    # scalar-queue position after all mm1 ACTs: the 2MB wv
                    # stream starts ~10us; first v use is ~35us.
                    issue_wv_loads()
                    warmup(2)
            if lo:
                for jt in range(1, JT):
                    mm2_jt(ch, st, jt)
            fst = prev
            prev = st if lo else None
        # drain the last pending f (chunk nch-1)
        if fst is not None:
            f_mm(fst)
            f_fin(fst)

    nc.compile()
    return nc


def _pack_k(m: np.ndarray) -> np.ndarray:
    """(1024, F) contraction-major -> (128, 8*F); tile kt at [:, kt*F:(kt+1)*F]."""
    kdim, f = m.shape
    assert kdim == H
    return np.ascontiguousarray(
        m.reshape(JT, 128, f).transpose(1, 0, 2).reshape(128, JT * f))


def _pack_k_outer(m: np.ndarray) -> np.ndarray:
    """(1024, 1024) contraction-major -> (128, 8*1024) with the OUTPUT tile
    index outer: tile (kt, jt) at [:, jt*1024 + kt*128]."""
    t = m.reshape(JT, 128, JT, 128).transpose(1, 2, 0, 3)  # (kp, jt, kt, jc)
    return np.ascontiguousarray(t.reshape(128, JT * H))


def _fwdcast(a: np.ndarray) -> np.ndarray:
    return a.astype(mybir.dt.np(FWD_DT))


def _bwdcast(a: np.ndarray) -> np.ndarray:
    return a.astype(mybir.dt.np(BWD_DT))


def prep_weights(w1, w1_2, b1, w2, b2, w3, b3):
    f = np.float32
    wh1 = np.ascontiguousarray(
        np.stack([w1[:, 0], w1[:, 1], w1_2[:, 0], w1_2[:, 1]]).astype(f))
    wh2 = _pack_k_outer(np.ascontiguousarray(w2.T).astype(f))  # lhsT[k,j]=w2[j,k]
    # v weights with the -4*w3[j] G-scale folded in: lhsT[j,i]=w2[j,i]*(-4 w3[j])
    wvs = (w2.astype(f) * (-4.0 * w3.reshape(-1, 1).astype(f)))
    wv = _pack_k_outer(wvs)
    wy = np.ascontiguousarray(w3.reshape(H).reshape(JT, 128).T.astype(f))
    wd4 = _pack_k(np.ascontiguousarray(
        np.stack([w1[:, 0], w1_2[:, 0], w1[:, 1], w1_2[:, 1]], axis=1)).astype(f))
    # widen each it-tile from 4 to 32 weight columns (28 zeros) so the d
    # matmuls write all 32 rows of each sm col group
    wd = np.zeros((128, 32 * JT), dtype=f)
    for it in range(JT):
        wd[:, 32 * it:32 * it + 4] = wd4[:, 4 * it:4 * (it + 1)]
    # f-matmul now consumes the raw d partials (sm): rows of sm are
    # [A1g, B1g, A2g, B2g] (A=w1-weighted, B=w1_2-weighted, per col group g).
    # pf rows 0:2 = [A1, -A2], rows 32:34 = [B1, -B2] (32-aligned partition
    # bases for the DVE finish: dd = pf[0:2]*[x0;x1] + pf[32:34]).
    wf = np.zeros((128, 128), dtype=f)
    for g in range(4):
        wf[32 * g + 0, 0] = 1.0    # A1
        wf[32 * g + 2, 1] = -1.0   # -A2
        wf[32 * g + 1, 32] = 1.0   # B1
        wf[32 * g + 3, 33] = -1.0  # -B2
    b1t = np.ascontiguousarray(b1.reshape(JT, 128).T.astype(f))
    b2t = np.ascontiguousarray(b2.reshape(JT, 128).T.astype(f))
    b3t = np.asarray(b3, dtype=f).reshape(1, 1)
    return _fwdcast(wh1), dict(wh2=_fwdcast(wh2), wv=_bwdcast(wv),
                               wy=wy, wd=_bwdcast(wd), wf=_bwdcast(wf),
                               b1t=b1t, b2t=b2t, b3t=b3t)


_PROG_CACHE: dict = {}


def _install_trace_support():
    """The agent image lacks the ``antenv.axon_hooks`` shim that the axon
    NTFF-profiling path imports; recreate it and register the ctypes hook.
    Also neuter ``upload_artifacts`` (zero-egress container)."""
    import sys
    import types
    try:
        import antenv.axon_hooks  # noqa: F401
    except ImportError:
        import antenv
        mod = types.ModuleType("antenv.axon_hooks")
        holder = {}
        mod.set_axon_ntff_profile_hook = lambda h: holder.__setitem__("h", h)
        mod.get_axon_ntff_profile_hook = lambda: holder.get("h")
        sys.modules["antenv.axon_hooks"] = mod
        antenv.axon_hooks = mod
        from trn_agent_boot.trn_boot import _ntff_profile_via_ctypes
        hook = _ntff_profile_via_ctypes("/opt/axon/libaxon_pjrt.so")
        if hook is not None:
            mod.set_axon_ntff_profile_hook(hook)
    import concourse.bass_utils as bu
    bu.upload_artifacts = lambda tmpdir: tmpdir


def kernel(x, w1, w1_2, b1, w2, b2, w3, b3, trace=False, _chunk=512):
    x = np.asarray(x, dtype=np.float32)
    wh1p, wdict = prep_weights(np.asarray(w1), np.asarray(w1_2),
                               np.asarray(b1), np.asarray(w2),
                               np.asarray(b2), np.asarray(w3),
                               np.asarray(b3))

    key = (NXL, _chunk)
    if key not in _PROG_CACHE:
        _PROG_CACHE[key] = build_program(NXL, _chunk)
    nc = _PROG_CACHE[key]

    in_maps = []
    for c in range(N_CORES):
        xs = x[c * NXL:(c + 1) * NXL]                 # (NXL, 2)
        x0, x1 = xs[:, 0].copy(), xs[:, 1].copy()
        xrs = _fwdcast(np.ascontiguousarray(np.concatenate([
            np.stack([x0 * x0, x1 * x1, x0, x1]), wh1p],
            axis=1)))                                 # (4, NXL + H)
        xqs = _bwdcast(np.ascontiguousarray(
            np.stack([x0, x1])))                      # (2, NXL)
        in_maps.append({"xr": xrs, "xq": xqs, **wdict})

    if trace:
        _install_trace_support()
    res = run_bass_kernel_spmd(nc, in_maps, core_ids=list(range(N_CORES)),
                               trace=trace)

    y = np.concatenate([res.results[c]["outy"].reshape(NXL)
                        for c in range(N_CORES)]).reshape(NX, 1)
    d2 = np.concatenate([res.results[c]["outdd"][1].reshape(NXL)
                         for c in range(N_CORES)]).reshape(NX, 1)
    m1 = np.concatenate([res.results[c]["outdd"][0].reshape(NXL)
                         for c in range(N_CORES)]).reshape(NX, 1)
    out = (y.astype(np.float32), d2.astype(np.float32), m1.astype(np.float32))
    if trace:
        return out, res
    return out

